# revision 2
# baseline (speedup 1.0000x reference)
"""Multi-head attention Trainium2 kernel (8 NeuronCores).

Sharding: core c = b*4 + g handles batch b (of 2) and head-group g (4 of the
16 heads). Q/K/V projections are column-sharded (256 cols per core), the
output projection is row-sharded; per-core partial outputs are summed on the
host (the all-reduce of a row-parallel matmul).

Per-core algorithm (all matmuls bf16 with f32 PSUM accumulation):
  Q.T = (WQg/8).T @ x_q.T           [256, 2048]  (scale 1/sqrt(D) folded in)
  K.T = WKg.T @ x_k.T               [256, 2048]
  V   = x_v @ WVg (+ ones column)   [2048, 4*65]
  S.T = Kh @ Qh.T per head          [Lk, Lq] tiles, 2 heads row-tiled on PE
  E.T = exp(S.T) * maskT            (multiplicative 0/1 mask == where(mask==0,-1e9) + softmax)
  Z.T|denom = V'.T @ E.T            M=65 matmul; row 64 = softmax denominator
  Z.T_norm = Z.T * bcast(1/denom)
  out_partial = Z.T_norm.T @ WOg    [2048, 1024] f32
Host: out[b] = sum_g out_partial[b,g] + bO.
"""

import sys
import types

sys.path.insert(0, "/opt/trn_rl_repo")

# The NTFF profiling hook module is absent in this container; shim it so
# run_bass_kernel_spmd(trace=True) degrades gracefully instead of crashing.
if "antenv.axon_hooks" not in sys.modules:
    _shim = types.ModuleType("antenv.axon_hooks")
    _shim.get_axon_ntff_profile_hook = lambda: None
    sys.modules["antenv.axon_hooks"] = _shim

import numpy as np
import ml_dtypes

import concourse.bass as bass
import concourse.mybir as mybir
import concourse.tile as tile
from concourse import bacc
from concourse.bass_utils import run_bass_kernel_spmd

BF16 = mybir.dt.bfloat16
F32 = mybir.dt.float32
AF = mybir.ActivationFunctionType
ALU = mybir.AluOpType

N_CORES = 8
B, L, C, H = 2, 2048, 1024, 16
D = C // H          # 64 head dim
G = 4               # head-groups per batch (cores per batch)
HPG = H // G        # 4 heads per group
DG = HPG * D        # 256 features per group
DP = D + 1          # head dim + ones column

P = 128
LQB = 512           # Lq block (psum free dim)
N_LQB = L // LQB    # 4
N_LK = L // P       # 16 Lk tiles
N_KT = C // P       # 8 contraction tiles for projections
MT_Q = DG // P      # 2 partition tiles for Q.T/K.T

_cached_nc = None


def _build():
    nc = bacc.Bacc("TRN2", target_bir_lowering=False, debug=False,
                   num_devices=N_CORES)

    xq = nc.dram_tensor("xq", [C, L], BF16, kind="ExternalInput").ap()
    xk = nc.dram_tensor("xk", [C, L], BF16, kind="ExternalInput").ap()
    xv = nc.dram_tensor("xv", [C, L], BF16, kind="ExternalInput").ap()
    wq = nc.dram_tensor("wq", [C, DG], BF16, kind="ExternalInput").ap()
    wk = nc.dram_tensor("wk", [C, DG], BF16, kind="ExternalInput").ap()
    wv = nc.dram_tensor("wv", [C, DG], BF16, kind="ExternalInput").ap()
    wo = nc.dram_tensor("wo", [DG, C], BF16, kind="ExternalInput").ap()
    bq = nc.dram_tensor("bq", [P, MT_Q], F32, kind="ExternalInput").ap()
    bk = nc.dram_tensor("bk", [P, MT_Q], F32, kind="ExternalInput").ap()
    bv = nc.dram_tensor("bv", [1, DG], BF16, kind="ExternalInput").ap()
    maskT = nc.dram_tensor("maskT", [L, L], BF16, kind="ExternalInput").ap()
    out = nc.dram_tensor("out", [L, C], F32, kind="ExternalOutput").ap()

    with tile.TileContext(nc) as tc:
        _body(tc, xq, xk, xv, wq, wk, wv, wo, bq, bk, bv, maskT, out)
    nc.compile()
    return nc


def _body(tc, xq, xk, xv, wq, wk, wv, wo, bq, bk, bv, maskT, out):
    nc = tc.nc
    from contextlib import ExitStack
    with ExitStack() as ctx:
        const = ctx.enter_context(tc.tile_pool(name="const", bufs=1))
        wpool = ctx.enter_context(tc.tile_pool(name="wpool", bufs=1))
        xvp = ctx.enter_context(tc.tile_pool(name="xvp", bufs=1))
        xs = ctx.enter_context(tc.tile_pool(name="xs", bufs=10))
        qk = ctx.enter_context(tc.tile_pool(name="qk", bufs=1))
        vpool = ctx.enter_context(tc.tile_pool(name="vpool", bufs=1))
        ztp = ctx.enter_context(tc.tile_pool(name="ztp", bufs=1))
        ep = ctx.enter_context(tc.tile_pool(name="ep", bufs=2))
        mp = ctx.enter_context(tc.tile_pool(name="mp", bufs=2))
        op = ctx.enter_context(tc.tile_pool(name="op", bufs=4))
        nrm = ctx.enter_context(tc.tile_pool(name="nrm", bufs=3))
        ps_big = ctx.enter_context(tc.tile_pool(name="ps_big", bufs=2, space="PSUM"))
        ps_av = ctx.enter_context(tc.tile_pool(name="ps_av", bufs=2, space="PSUM"))

        # ---- resident constants / weights ----
        wq_sb = wpool.tile([P, N_KT, DG], BF16)
        nc.sync.dma_start(wq_sb[:], wq.rearrange("(kt p) m -> p kt m", p=P))
        wk_sb = wpool.tile([P, N_KT, DG], BF16)
        nc.sync.dma_start(wk_sb[:], wk.rearrange("(kt p) m -> p kt m", p=P))
        wv_sb = wpool.tile([P, N_KT, DG], BF16)
        nc.sync.dma_start(wv_sb[:], wv.rearrange("(kt p) m -> p kt m", p=P))
        wo_sb = wpool.tile([P, MT_Q, C], BF16)
        nc.sync.dma_start(wo_sb[:], wo.rearrange("(kt p) n -> p kt n", p=P))
        bq_sb = const.tile([P, MT_Q], F32)
        nc.sync.dma_start(bq_sb[:], bq[:])
        bk_sb = const.tile([P, MT_Q], F32)
        nc.sync.dma_start(bk_sb[:], bk[:])
        bv_sb = const.tile([1, DG], BF16)
        nc.sync.dma_start(bv_sb[:], bv[:])
        ones_sb = const.tile([1, P], BF16)
        nc.gpsimd.memset(ones_sb[:], 1.0)

        # xv resident [128, 8, 2048] (lhsT tiles for the V projection)
        xv_sb = xvp.tile([P, N_KT, L], BF16)
        nc.sync.dma_start(xv_sb[:], xv.rearrange("(kt p) l -> p kt l", p=P))

        # ---- Q.T / K.T projections ----
        qt_sb = qk.tile([P, MT_Q, L], BF16)
        kt_sb = qk.tile([P, MT_Q, L], BF16)
        for (x_dram, w_sb, b_sb, dst) in (
            (xq, wq_sb, bq_sb, qt_sb),
            (xk, wk_sb, bk_sb, kt_sb),
        ):
            for lq in range(N_LQB):
                xtiles = []
                for kt in range(N_KT):
                    xt = xs.tile([P, LQB], BF16, tag="xs", name=f"xs_{kt}")
                    nc.sync.dma_start(
                        xt[:], x_dram[kt * P:(kt + 1) * P, lq * LQB:(lq + 1) * LQB])
                    xtiles.append(xt)
                for mt in range(MT_Q):
                    ps = ps_big.tile([P, LQB], F32, tag="ps", name="ps_proj")
                    for kt in range(N_KT):
                        nc.tensor.matmul(
                            ps[:], w_sb[:, kt, mt * P:(mt + 1) * P],
                            xtiles[kt][:],
                            start=(kt == 0), stop=(kt == N_KT - 1))
                    # psum -> sbuf bf16 with per-partition bias
                    nc.scalar.activation(
                        dst[:, mt, lq * LQB:(lq + 1) * LQB], ps[:],
                        AF.Identity, bias=b_sb[:, mt:mt + 1])

        # ---- V projection (natural layout, per-head padded with ones col) ----
        v_sb = vpool.tile([P, N_LK, HPG, DP], BF16)
        # ones columns (written once; tile has bufs=1 so it is never recycled)
        nc.gpsimd.memset(v_sb[:, :, :, D:DP], 1.0)
        for mt in range(N_LK):
            ps = ps_big.tile([P, DG], F32, tag="ps", name="ps_v")
            for kt in range(N_KT):
                nc.tensor.matmul(
                    ps[:], xv_sb[:, kt, mt * P:(mt + 1) * P], wv_sb[:, kt, :],
                    start=(kt == 0), stop=False)
            # bias via rank-1 update: ones[1,128].T @ bv[1,256]
            nc.tensor.matmul(ps[:], ones_sb[:], bv_sb[:], start=False, stop=True)
            # evict: [128, 4, 64] strided into the padded per-head layout
            nc.vector.tensor_copy(
                v_sb[:, mt, :, 0:D],
                ps[:].rearrange("p (h d) -> p h d", h=HPG))

        # ---- attention ----
        zt_sb = ztp.tile([P, MT_Q, L], BF16)
        for lq in range(N_LQB):
            m_sb = mp.tile([P, N_LK, LQB], BF16, tag="mask")
            nc.sync.dma_start(
                m_sb[:], maskT[:, lq * LQB:(lq + 1) * LQB]
                .rearrange("(lk p) q -> p lk q", p=P))
            for pair in range(MT_Q):
                e_sb = ep.tile([P, N_LK, 2, LQB], BF16, tag="e")
                # scores + exp, two heads row-tiled per lk tile
                for lk in range(N_LK):
                    ps = ps_big.tile([P, 2 * LQB], F32, tag="ps", name="ps_s")
                    nc.tensor.matmul(
                        ps[:, 0:LQB],
                        kt_sb[0:D, pair, lk * P:(lk + 1) * P],
                        qt_sb[0:D, pair, lq * LQB:(lq + 1) * LQB],
                        start=True, stop=True)
                    nc.tensor.matmul(
                        ps[:, LQB:2 * LQB],
                        kt_sb[D:P, pair, lk * P:(lk + 1) * P],
                        qt_sb[D:P, pair, lq * LQB:(lq + 1) * LQB],
                        start=True, stop=True)
                    nc.scalar.activation(e_sb[:, lk, :, :], ps[:], AF.Exp)
                # multiplicative mask (0/1), both heads
                for hh in range(2):
                    nc.vector.tensor_tensor(
                        e_sb[:, :, hh, :], e_sb[:, :, hh, :], m_sb[:],
                        ALU.mult)
                # A@V with fused denominator (ones column of V')
                for hh in range(2):
                    h = pair * 2 + hh
                    ps_z = ps_av.tile([P, LQB], F32, tag="av")
                    for lk in range(N_LK):
                        nc.tensor.matmul(
                            ps_z[0:DP, :],
                            v_sb[:, lk, h, :],
                            e_sb[:, lk, hh, :],
                            start=(lk == 0), stop=(lk == N_LK - 1))
                    # normalize: Z.T[d, q] / denom[q]
                    recip = nrm.tile([1, LQB], F32, tag="recip")
                    nc.vector.reciprocal(recip[:], ps_z[D:DP, :])
                    bcast = nrm.tile([D, LQB], F32, tag="bcast")
                    nc.gpsimd.partition_broadcast(bcast[:], recip[:])
                    nc.vector.tensor_tensor(
                        zt_sb[hh * D:(hh + 1) * D, pair,
                              lq * LQB:(lq + 1) * LQB],
                        ps_z[0:D, :], bcast[:], ALU.mult)

        # ---- output projection (row-parallel partial) ----
        for mt in range(N_LK):
            for nb in range(2):
                ps = ps_big.tile([P, LQB], F32, tag="ps", name="ps_o")
                for kt in range(MT_Q):
                    nc.tensor.matmul(
                        ps[:], zt_sb[:, kt, mt * P:(mt + 1) * P],
                        wo_sb[:, kt, nb * LQB:(nb + 1) * LQB],
                        start=(kt == 0), stop=(kt == MT_Q - 1))
                o_sb = op.tile([P, LQB], F32, tag="o")
                nc.vector.tensor_copy(o_sb[:], ps[:])
                nc.sync.dma_start(
                    out[mt * P:(mt + 1) * P, nb * LQB:(nb + 1) * LQB], o_sb[:])


def get_nc():
    global _cached_nc
    if _cached_nc is None:
        _cached_nc = _build()
    return _cached_nc


def _bf16(x):
    return np.asarray(x, dtype=np.float32).astype(ml_dtypes.bfloat16)


def kernel(**inputs):
    query = np.asarray(inputs["query"], np.float32)
    key = np.asarray(inputs["key"], np.float32)
    value = np.asarray(inputs["value"], np.float32)
    mask = np.asarray(inputs["mask"])
    WQ = np.asarray(inputs["WQ"], np.float32)
    bQ = np.asarray(inputs["bQ"], np.float32)
    WK = np.asarray(inputs["WK"], np.float32)
    bK = np.asarray(inputs["bK"], np.float32)
    WV = np.asarray(inputs["WV"], np.float32)
    bV = np.asarray(inputs["bV"], np.float32)
    WO = np.asarray(inputs["WO"], np.float32)
    bO = np.asarray(inputs["bO"], np.float32)

    nc = get_nc()

    scale = 1.0 / np.sqrt(np.float32(D))
    # per-batch host prep (shared across the 4 cores of a batch)
    xqT = [_bf16(query[b].T) for b in range(B)]
    xkT = [_bf16(key[b].T) for b in range(B)]
    xvT = [_bf16(value[b].T) for b in range(B)]
    maskTb = [_bf16(mask[b, 0].T) for b in range(B)]
    in_maps = []
    for c in range(N_CORES):
        b, g = divmod(c, G)
        sl = slice(g * DG, (g + 1) * DG)
        in_maps.append({
            "xq": xqT[b], "xk": xkT[b], "xv": xvT[b],
            "wq": _bf16(WQ[:, sl] * scale),
            "wk": _bf16(WK[:, sl]),
            "wv": _bf16(WV[:, sl]),
            "wo": _bf16(WO[sl, :]),
            "bq": np.ascontiguousarray(
                (bQ[sl] * scale).reshape(MT_Q, P).T).astype(np.float32),
            "bk": np.ascontiguousarray(
                bK[sl].reshape(MT_Q, P).T).astype(np.float32),
            "bv": _bf16(bV[sl]).reshape(1, DG),
            "maskT": maskTb[b],
        })

    res = run_bass_kernel_spmd(nc, in_maps, core_ids=list(range(N_CORES)))

    outp = np.zeros((B, L, C), np.float32)
    for c in range(N_CORES):
        b = c // G
        outp[b] += res.results[c]["out"]
    outp += bO.astype(np.float32)
    return outp


# revision 21
# speedup vs baseline: 1.2119x; 1.2119x over previous
"""Multi-head attention Trainium2 kernel (8 NeuronCores).

Sharding: core c = b*4 + g handles batch b (of 2) and head-group g (4 of the
16 heads). Q/K/V projections are column-sharded (256 cols per core), the
output projection is row-sharded; per-core partial outputs are summed on the
host (the all-reduce of a row-parallel matmul).

Per-core algorithm (all matmuls bf16 with f32 PSUM accumulation):
  Q.T = (WQg/8).T @ x_q.T           [256, 2048]  (scale 1/sqrt(D) folded in)
  K.T = WKg.T @ x_k.T               [256, 2048]
  V   = x_v @ WVg (+ ones column)   [2048, 4*65]
  S.T = Kh @ Qh.T per head          [Lk, Lq] tiles, 2 heads row-tiled on PE
  E.T = exp(S.T) * maskT            (multiplicative 0/1 mask == where(mask==0,-1e9) + softmax)
  Z.T|denom = V'.T @ E.T            M=65 matmul; row 64 = softmax denominator
  Z.T_norm = Z.T * bcast(1/denom)
  out_partial = Z.T_norm.T @ WOg    [2048, 1024] f32
Host: out[b] = sum_g out_partial[b,g] + bO.
"""

import sys
import types

sys.path.insert(0, "/opt/trn_rl_repo")

# The NTFF profiling hook module is absent in this container; shim it so
# run_bass_kernel_spmd(trace=True) degrades gracefully instead of crashing.
if "antenv.axon_hooks" not in sys.modules:
    _shim = types.ModuleType("antenv.axon_hooks")
    _shim.get_axon_ntff_profile_hook = lambda: None
    sys.modules["antenv.axon_hooks"] = _shim

import numpy as np
import ml_dtypes

import concourse.bass as bass
import concourse.mybir as mybir
import concourse.tile as tile
from concourse import bacc
from concourse.bass_utils import run_bass_kernel_spmd

BF16 = mybir.dt.bfloat16
F32 = mybir.dt.float32
AF = mybir.ActivationFunctionType
ALU = mybir.AluOpType

N_CORES = 8
B, L, C, H = 2, 2048, 1024, 16
D = C // H          # 64 head dim
G = 4               # head-groups per batch (cores per batch)
HPG = H // G        # 4 heads per group
DG = HPG * D        # 256 features per group
DP = D + 1          # head dim + ones column

P = 128
LQB = 512           # Lq block (psum free dim)
N_LQB = L // LQB    # 4
N_LK = L // P       # 16 Lk tiles
N_KT = C // P       # 8 contraction tiles for projections
MT_Q = DG // P      # 2 partition tiles for Q.T/K.T

_cached_nc = None


def _build():
    nc = bacc.Bacc("TRN2", target_bir_lowering=False, debug=False,
                   num_devices=N_CORES)

    xq = nc.dram_tensor("xq", [C, L], BF16, kind="ExternalInput").ap()
    xk = nc.dram_tensor("xk", [C, L], BF16, kind="ExternalInput").ap()
    xv = nc.dram_tensor("xv", [C, L], BF16, kind="ExternalInput").ap()
    wq = nc.dram_tensor("wq", [C, DG], BF16, kind="ExternalInput").ap()
    wk = nc.dram_tensor("wk", [C, DG], BF16, kind="ExternalInput").ap()
    wv = nc.dram_tensor("wv", [C, DG], BF16, kind="ExternalInput").ap()
    wo = nc.dram_tensor("wo", [DG, C], BF16, kind="ExternalInput").ap()
    bq = nc.dram_tensor("bq", [P, MT_Q], F32, kind="ExternalInput").ap()
    bk = nc.dram_tensor("bk", [P, MT_Q], F32, kind="ExternalInput").ap()
    bv = nc.dram_tensor("bv", [1, DG], BF16, kind="ExternalInput").ap()
    maskT = nc.dram_tensor("maskT", [L, L], BF16, kind="ExternalInput").ap()
    out = nc.dram_tensor("out", [L, C], F32, kind="ExternalOutput").ap()

    with tile.TileContext(nc) as tc:
        _body(tc, xq, xk, xv, wq, wk, wv, wo, bq, bk, bv, maskT, out)
    nc.compile()
    return nc


def _body(tc, xq, xk, xv, wq, wk, wv, wo, bq, bk, bv, maskT, out):
    import os
    PHASE = int(os.environ.get("K_PHASE", "99"))  # debug: truncate build
    nc = tc.nc
    from contextlib import ExitStack
    with ExitStack() as ctx:
        const = ctx.enter_context(tc.tile_pool(name="const", bufs=1))
        wpool = ctx.enter_context(tc.tile_pool(name="wpool", bufs=1))
        xvp = ctx.enter_context(tc.tile_pool(name="xvp", bufs=3))
        xs = ctx.enter_context(tc.tile_pool(name="xs", bufs=3))
        qk = ctx.enter_context(tc.tile_pool(name="qk", bufs=1))
        vpool = ctx.enter_context(tc.tile_pool(name="vpool", bufs=1))
        ztp = ctx.enter_context(tc.tile_pool(name="ztp", bufs=1))
        ep = ctx.enter_context(tc.tile_pool(name="ep", bufs=2))
        mp = ctx.enter_context(tc.tile_pool(name="mp", bufs=2))
        op = ctx.enter_context(tc.tile_pool(name="op", bufs=4))
        nrm = ctx.enter_context(tc.tile_pool(name="nrm", bufs=3))
        ps_big = ctx.enter_context(tc.tile_pool(name="ps_big", bufs=2, space="PSUM"))
        ps_av = ctx.enter_context(tc.tile_pool(name="ps_av", bufs=2, space="PSUM"))
        ps_op = ctx.enter_context(tc.tile_pool(name="ps_op", bufs=2, space="PSUM"))

        # ---- resident constants / weights ----
        wq_sb = wpool.tile([P, N_KT, DG], BF16)
        nc.sync.dma_start(wq_sb[:], wq.rearrange("(kt p) m -> p kt m", p=P))
        wk_sb = wpool.tile([P, N_KT, DG], BF16)
        nc.sync.dma_start(wk_sb[:], wk.rearrange("(kt p) m -> p kt m", p=P))
        wv_sb = wpool.tile([P, N_KT, DG], BF16)
        wo_sb = wpool.tile([P, MT_Q, C], BF16)
        bq_sb = const.tile([P, MT_Q], F32)
        nc.sync.dma_start(bq_sb[:], bq[:])
        bk_sb = const.tile([P, MT_Q], F32)
        nc.sync.dma_start(bk_sb[:], bk[:])
        bv_sb = const.tile([1, DG], BF16)
        nc.sync.dma_start(bv_sb[:], bv[:])
        ones_sb = const.tile([1, P], BF16)
        nc.gpsimd.memset(ones_sb[:], 1.0)



        # ---- projection emitters ----
        qt_sb = qk.tile([P, MT_Q, L], BF16)
        kt_sb = qk.tile([P, MT_Q, L], BF16)
        v_sb = vpool.tile([P, N_LK, HPG, DP], BF16)
        # ones columns (written once; tile has bufs=1 so it is never recycled)
        nc.gpsimd.memset(v_sb[:, :, :, D:DP], 1.0)

        def emit_qk_proj(mt):
            # Q.T / K.T rows [mt*128:(mt+1)*128] — enough for scores pair mt
            for (x_dram, w_sb, b_sb, dst) in (
                (xq, wq_sb, bq_sb, qt_sb),
                (xk, wk_sb, bk_sb, kt_sb),
            ):
                xr = x_dram.rearrange("(kt p) l -> p kt l", p=P)
                for lq in range(N_LQB):
                    xt = xs.tile([P, N_KT, LQB], BF16, tag="xs", name="xs_t")
                    nc.sync.dma_start(
                        xt[:], xr[:, :, lq * LQB:(lq + 1) * LQB])
                    ps = ps_big.tile([P, LQB], F32, tag="ps", name="ps_proj")
                    for kt in range(N_KT):
                        nc.tensor.matmul(
                            ps[:], w_sb[:, kt, mt * P:(mt + 1) * P],
                            xt[:, kt, :],
                            start=(kt == 0), stop=(kt == N_KT - 1))
                    # psum -> sbuf bf16 with per-partition bias (DVE,
                    # keeping ACT free for the exp stream)
                    nc.vector.tensor_scalar_add(
                        dst[:, mt, lq * LQB:(lq + 1) * LQB], ps[:],
                        b_sb[:, mt:mt + 1])

        def emit_v_proj():
            nc.sync.dma_start(wv_sb[:], wv.rearrange("(kt p) m -> p kt m", p=P))
            xvr = xv.rearrange("(kt p) l -> p kt l", p=P)
            for lb in range(N_LQB):
                xt = xs.tile([P, N_KT, LQB], BF16, tag="xs", name="xs_t")
                nc.sync.dma_start(xt[:], xvr[:, :, lb * LQB:(lb + 1) * LQB])
                for sub in range(LQB // P):
                    mt = lb * (LQB // P) + sub
                    ps = ps_big.tile([P, DG], F32, tag="ps", name="ps_v")
                    for kt in range(N_KT):
                        nc.tensor.matmul(
                            ps[:], xt[:, kt, sub * P:(sub + 1) * P],
                            wv_sb[:, kt, :],
                            start=(kt == 0), stop=False)
                    # bias via rank-1 update: ones[1,128].T @ bv[1,256]
                    nc.tensor.matmul(ps[:], ones_sb[:], bv_sb[:],
                                     start=False, stop=True)
                    # evict: [128, 4, 64] strided into the padded layout
                    nc.vector.tensor_copy(
                        v_sb[:, mt, :, 0:D],
                        ps[:].rearrange("p (h d) -> p h d", h=HPG))

        if PHASE < 2:
            emit_qk_proj(0)
            emit_qk_proj(1)
            emit_v_proj()
            return
        # ---- attention (software-pipelined: scores(i+1) emitted before
        # AV(i) so the in-order PE stream never stalls on exp/mask) ----
        zt_sb = ztp.tile([P, MT_Q, L], BF16)
        mask_tiles = {}

        def emit_scores(lq, pair):
            if lq not in mask_tiles:
                m_sb = mp.tile([P, N_LK, LQB], BF16, tag="mask", name="m_sb")
                nc.sync.dma_start(
                    m_sb[:], maskT[:, lq * LQB:(lq + 1) * LQB]
                    .rearrange("(lk p) q -> p lk q", p=P))
                mask_tiles[lq] = m_sb
            e_sb = ep.tile([P, N_LK, 2, LQB], BF16, tag="e", name="e_sb")
            # scores + exp, two heads row-tiled per lk tile
            for lk in range(N_LK):
                ps = ps_big.tile([P, 2 * LQB], F32, tag="ps", name="ps_s")
                nc.tensor.matmul(
                    ps[:, 0:LQB],
                    kt_sb[0:D, pair, lk * P:(lk + 1) * P],
                    qt_sb[0:D, pair, lq * LQB:(lq + 1) * LQB],
                    start=True, stop=True)
                nc.tensor.matmul(
                    ps[:, LQB:2 * LQB],
                    kt_sb[D:P, pair, lk * P:(lk + 1) * P],
                    qt_sb[D:P, pair, lq * LQB:(lq + 1) * LQB],
                    start=True, stop=True)
                nc.scalar.activation(e_sb[:, lk, :, :], ps[:], AF.Exp)
            # multiplicative mask (0/1), both heads
            for hh in range(2):
                nc.vector.tensor_tensor(
                    e_sb[:, :, hh, :], e_sb[:, :, hh, :], mask_tiles[lq][:],
                    ALU.mult)
            return e_sb

        def emit_av(lq, pair, e_sb):
            # A@V with fused denominator (ones column of V')
            for hh in range(2):
                h = pair * 2 + hh
                ps_z = ps_av.tile([P, LQB], F32, tag="av", name="ps_z")
                for lk in range(N_LK):
                    nc.tensor.matmul(
                        ps_z[0:DP, :],
                        v_sb[:, lk, h, :],
                        e_sb[:, lk, hh, :],
                        start=(lk == 0), stop=(lk == N_LK - 1))
                # normalize: Z.T[d, q] / denom[q]
                recip = nrm.tile([1, LQB], F32, tag="recip", name="recip")
                nc.vector.reciprocal(recip[:], ps_z[D:DP, :])
                bcast = nrm.tile([D, LQB], F32, tag="bcast", name="bcast")
                nc.gpsimd.partition_broadcast(bcast[:], recip[:])
                nc.vector.tensor_tensor(
                    zt_sb[hh * D:(hh + 1) * D, pair,
                          lq * LQB:(lq + 1) * LQB],
                    ps_z[0:D, :], bcast[:], ALU.mult)

        wo_loaded = [False]

        def emit_outproj(lq):
            # partial output rows for this lq block (needs both pairs' Z.T)
            if not wo_loaded[0]:
                nc.sync.dma_start(
                    wo_sb[:], wo.rearrange("(kt p) n -> p kt n", p=P))
                wo_loaded[0] = True
            for sub in range(LQB // P):
                mt = lq * (LQB // P) + sub
                o_sb = op.tile([P, C], F32, tag="o", name="o_sb")
                for nb in range(2):
                    ps = ps_op.tile([P, LQB], F32, tag="ps_o", name="ps_o")
                    for kt in range(MT_Q):
                        nc.tensor.matmul(
                            ps[:], zt_sb[:, kt, mt * P:(mt + 1) * P],
                            wo_sb[:, kt, nb * LQB:(nb + 1) * LQB],
                            start=(kt == 0), stop=(kt == MT_Q - 1))
                    nc.vector.tensor_copy(
                        o_sb[:, nb * LQB:(nb + 1) * LQB], ps[:])
                nc.sync.dma_start(out[mt * P:(mt + 1) * P, :], o_sb[:])

        # emission order: enough projection for the first scores blocks,
        # then a lag-1 pipeline of scores -> AV, with the output projection
        # of each lq block interleaved once both its pairs are normalized.
        ORDER = int(os.environ.get("K_ORDER", "1"))
        interleave_outproj = ORDER in (1, 2)

        def attn_pipeline(start_pending):
            pending = start_pending
            first = 2 if pending else 0
            for i in range(first, N_LQB * MT_Q):
                lq, pair = divmod(i, MT_Q)
                e_sb = emit_scores(lq, pair)
                if pending:
                    blq, bpair, be = pending.pop(0)
                    emit_av(blq, bpair, be)
                    if interleave_outproj and bpair == MT_Q - 1:
                        emit_outproj(blq)
                pending.append((lq, pair, e_sb))
            for (blq, bpair, be) in pending:
                emit_av(blq, bpair, be)
                if interleave_outproj and bpair == MT_Q - 1:
                    emit_outproj(blq)
            if not interleave_outproj:
                for lq in range(N_LQB):
                    emit_outproj(lq)

        if ORDER == 2:
            emit_qk_proj(0)
            e00 = emit_scores(0, 0)
            emit_qk_proj(1)
            e01 = emit_scores(0, 1)
            emit_v_proj()
            emit_av(0, 0, e00)
            attn_pipeline([(0, 1, e01)])
        else:
            emit_qk_proj(0)
            emit_qk_proj(1)
            emit_v_proj()
            attn_pipeline([])


def get_nc():
    global _cached_nc
    if _cached_nc is None:
        _cached_nc = _build()
    return _cached_nc


def _bf16(x):
    return np.asarray(x, dtype=np.float32).astype(ml_dtypes.bfloat16)


def kernel(**inputs):
    query = np.asarray(inputs["query"], np.float32)
    key = np.asarray(inputs["key"], np.float32)
    value = np.asarray(inputs["value"], np.float32)
    mask = np.asarray(inputs["mask"])
    WQ = np.asarray(inputs["WQ"], np.float32)
    bQ = np.asarray(inputs["bQ"], np.float32)
    WK = np.asarray(inputs["WK"], np.float32)
    bK = np.asarray(inputs["bK"], np.float32)
    WV = np.asarray(inputs["WV"], np.float32)
    bV = np.asarray(inputs["bV"], np.float32)
    WO = np.asarray(inputs["WO"], np.float32)
    bO = np.asarray(inputs["bO"], np.float32)

    nc = get_nc()

    scale = 1.0 / np.sqrt(np.float32(D))
    # per-batch host prep (shared across the 4 cores of a batch)
    xqT = [_bf16(query[b].T) for b in range(B)]
    xkT = [_bf16(key[b].T) for b in range(B)]
    xvT = [_bf16(value[b].T) for b in range(B)]
    maskTb = [_bf16(mask[b, 0].T) for b in range(B)]
    in_maps = []
    for c in range(N_CORES):
        b, g = divmod(c, G)
        sl = slice(g * DG, (g + 1) * DG)
        in_maps.append({
            "xq": xqT[b], "xk": xkT[b], "xv": xvT[b],
            "wq": _bf16(WQ[:, sl] * scale),
            "wk": _bf16(WK[:, sl]),
            "wv": _bf16(WV[:, sl]),
            "wo": _bf16(WO[sl, :]),
            "bq": np.ascontiguousarray(
                (bQ[sl] * scale).reshape(MT_Q, P).T).astype(np.float32),
            "bk": np.ascontiguousarray(
                bK[sl].reshape(MT_Q, P).T).astype(np.float32),
            "bv": _bf16(bV[sl]).reshape(1, DG),
            "maskT": maskTb[b],
        })

    res = run_bass_kernel_spmd(nc, in_maps, core_ids=list(range(N_CORES)))

    outp = np.zeros((B, L, C), np.float32)
    for c in range(N_CORES):
        b = c // G
        outp[b] += res.results[c]["out"]
    outp += bO.astype(np.float32)
    return outp
